# revision 25
# baseline (speedup 1.0000x reference)
"""BitLinear (ternary weight quantization + linear) on 8 TRN2 NeuronCores.

y = x @ w_eff.T with w_eff = clip(round(w/scale), -1, 1) * scale,
scale = clamp(mean |w| per row, 1e-5).

Sharding: column-parallel — weight rows (out_features) split 8 ways; each
core computes y[:, shard] for the full x; host concatenates. Quantization
is per-output-row, so it is fully local to a shard.

v2 dataflow (vs the transpose-on-device baseline):
  * The host pre-permutes x into [16 chunks][128 k_in][16 k_sub][512 rows]
    so every x tile lands in SBUF already in stationary (k-major) layout —
    the device does ZERO x preprocessing (the baseline spent ~86us of PE
    time on 1024 PE transposes plus DVE/ACT copies for this).
  * The matmul runs in bf16: w_eff is ternary*scale (exact in bf16 up to a
    coherent 0.4% per-row scale rounding), x is cast fp32->bf16 in-flight
    by the SWDGE DMA. bf16 enables fast weight load so the per-matmul
    LDWEIGHTS (~187ns in fp32r) hides completely under the 213ns matmul.
  * W phase keeps the baseline's quantization math bit-identical (Abs
    row-sum on ACT, scale clamp, is_gt/is_lt ternary build on DVE): the
    jax reference's round() at the 0.5 boundary is reproduced exactly —
    a single flipped ternary weight costs 1.35e-2 absmax error, 2/3 of
    the 2e-2 budget.
  * Warm-up matmuls at kernel start bring the PE HAM clock gate to 8/8
    before the real matmul stream begins.

Per-core steady state: 16 row-chunks of 512; per chunk one 4MiB cast-DMA
in, then 8 accumulation groups (4 m-subtiles x 2 n-slices) of 16 matmuls
[128x128]@[128x512], ACT eviction, 256KB y DMA out per group.
"""

import ml_dtypes
import numpy as np

import concourse.bass as bass
import concourse.mybir as mybir
import concourse.tile as tile
from concourse import bacc
from concourse.bass_utils import run_bass_kernel_spmd
from concourse.masks import make_identity

F32 = mybir.dt.float32
F32R = mybir.dt.float32r
BF16 = mybir.dt.bfloat16

# Problem shape (hardcoded per contract)
B, S, D_IN, D_OUT = 4, 2048, 2048, 8192
NCORES = 8
R = B * S                 # 8192 rows of x
O = D_OUT // NCORES       # 1024 out features per core
K_SUB = D_IN // 128       # 16 contraction sub-tiles
O_TILES = O // 128        # 8 weight row-tiles per core
N_SLICE = 512             # psum bank width (fp32)
N_SLICES = O // N_SLICE   # 2
TGRP = 4                  # transposes batched per psum bank
RCHUNK = 512              # x rows per streamed chunk
NCHUNK = R // RCHUNK      # 16
MSUB = RCHUNK // 128      # 4
N_WARM = 20               # HAM warm-up matmuls


def _build():
    nc = bacc.Bacc(None, target_bir_lowering=False)

    # x: host-permuted bf16 [chunk, k_in, msub, k_sub, row] (full x, same
    # on all cores); one 512KiB sub-DMA per (chunk, msub) keeps
    # dependencies fine-grained so the first matmul group only waits for
    # its own slice. Shipping bf16 halves x HBM traffic (the early phase
    # is HBM-bound: w + x + y competing) and avoids the SWDGE cast path.
    x_d = nc.dram_tensor("x", [NCHUNK, 128, MSUB, K_SUB, 128], BF16,
                         kind="ExternalInput")
    w_d = nc.dram_tensor("w", [O, D_IN], F32, kind="ExternalInput")
    y_d = nc.dram_tensor("y", [R, O], F32, kind="ExternalOutput")

    with tile.TileContext(nc) as tc:
        with (
            tc.tile_pool(name="const", bufs=1) as const,
            tc.tile_pool(name="wt", bufs=1) as wtp,
            tc.tile_pool(name="ws", bufs=1) as ws,
            tc.tile_pool(name="xs", bufs=1) as xs,
            tc.tile_pool(name="ys", bufs=1) as ysp,
            tc.tile_pool(name="ps", bufs=2, space="PSUM") as ps,
            tc.tile_pool(name="ymm", bufs=1, space="PSUM") as ymm,
        ):
            # HAM warm-up: keep the PE busy with throwaway matmuls during
            # the W-phase lead-in so the clock gate is at 8/8 when the
            # real stream starts. (PE transposes don't count as HAM-busy.)
            dummy = const.tile([128, N_SLICE], BF16)
            nc.vector.memset(dummy[:], 0.0)
            wacc = ymm.tile([128, N_SLICE], F32, tag="warm", bufs=1)

            def warmup(n):
                for _ in range(n):
                    nc.tensor.matmul(wacc[:], dummy[:, :128], dummy[:],
                                     start=True, stop=True)

            ident_f = const.tile([128, 128], F32)
            make_identity(nc, ident_f[:])
            ident = const.tile([128, 128], F32R)
            nc.vector.tensor_copy(ident[:], ident_f[:])

            # W^T resident in SBUF (bf16), one tile per n-slice:
            # wts[n][:, k, o'] = w_eff^T[k_in, k_sub, n*512 + o']
            wts = [
                wtp.tile([128, K_SUB, N_SLICE], BF16, name=f"wt{n}")
                for n in range(N_SLICES)
            ]

            w_tiles = {}

            def w_dma(a):
                """Start the DMA for weight rows a*128..(a+1)*128.

                Alternates between the two HWDGE rings so the 8 transfers
                overlap pairwise (each ring executes its DMAs in FIFO).
                """
                w_in = ws.tile([128, D_IN], F32, tag="w_in", bufs=4,
                               name=f"w_in_{a}")
                eng = nc.sync if a % 2 == 0 else nc.scalar
                eng.dma_start(w_in[:], w_d[a * 128 : (a + 1) * 128, :])
                w_tiles[a] = w_in

            def w_quant(a):
                """Quantize + transpose weight rows a*128..(a+1)*128.

                Math is bit-identical to the baseline (matches the jax
                reference's round-half behavior at the 0.5 boundary); only
                the final PSUM->SBUF eviction casts to bf16.
                """
                w_in = w_tiles.pop(a)

                absdump = ws.tile([128, D_IN], F32, tag="w_neg",
                                  name=f"absdump_{a}")
                ssum = ws.tile([128, 1], F32, tag="w_sum", name=f"ssum_{a}")
                nc.scalar.activation(
                    absdump[:], w_in[:],
                    mybir.ActivationFunctionType.Abs,
                    accum_out=ssum[:],
                )
                scale = ws.tile([128, 1], F32, tag="w_scale",
                                name=f"scale_{a}")
                nc.vector.tensor_scalar(
                    out=scale[:], in0=ssum[:], scalar1=1.0 / D_IN,
                    scalar2=1e-5, op0=mybir.AluOpType.mult,
                    op1=mybir.AluOpType.max,
                )
                hpos = ws.tile([128, 1], F32, tag="w_hpos", name=f"hp_{a}")
                hneg = ws.tile([128, 1], F32, tag="w_hneg", name=f"hn_{a}")
                nc.vector.tensor_scalar_mul(hpos[:], scale[:], 0.5)
                nc.vector.tensor_scalar_mul(hneg[:], scale[:], -0.5)

                # (w > 0.5*scale)*scale - (w < -0.5*scale)*scale
                # pos on DVE, neg on GPSIMD in parallel: the DVE-serial
                # quant chain is the wts[] critical path during lead-in.
                # Both are exact compare+mult ops, so engine choice does
                # not affect the 0.5-boundary math.
                pos = ws.tile([128, D_IN], F32, tag="w_pos", name=f"pos_{a}")
                nc.vector.tensor_scalar(
                    out=pos[:], in0=w_in[:], scalar1=hpos[:], scalar2=scale[:],
                    op0=mybir.AluOpType.is_gt, op1=mybir.AluOpType.mult,
                )
                neg = ws.tile([128, D_IN], F32, tag="w_neg", name=f"neg_{a}")
                nc.gpsimd.tensor_scalar(
                    out=neg[:], in0=w_in[:], scalar1=hneg[:], scalar2=scale[:],
                    op0=mybir.AluOpType.is_lt, op1=mybir.AluOpType.mult,
                )
                weff = ws.tile([128, D_IN], F32R, tag="w_eff",
                               name=f"weff_{a}")
                nc.vector.tensor_sub(weff[:], pos[:], neg[:])

                n_idx, o_off = divmod(a * 128, N_SLICE)
                for kg in range(K_SUB // TGRP):
                    pt = ps.tile([128, TGRP * 128], F32, tag="wtps", bufs=2,
                                 name=f"wpt_{a}_{kg}")
                    for j in range(TGRP):
                        k = kg * TGRP + j
                        nc.tensor.transpose(
                            pt[:, j * 128 : (j + 1) * 128].bitcast(F32R),
                            weff[:, k * 128 : (k + 1) * 128],
                            ident[:],
                        )
                    half = TGRP // 2
                    dst = wts[n_idx][:, kg * TGRP : (kg + 1) * TGRP,
                                     o_off : o_off + 128]
                    src = pt[:].rearrange("p (g c) -> p g c", g=TGRP)
                    nc.scalar.copy(dst[:, :half], src[:, :half])
                    nc.scalar.copy(dst[:, half:], src[:, half:])

            def x_load(c):
                """Start 4 per-msub SWDGE DMAs for x chunk c (512KiB each).

                On the gpsimd (SWDGE) path: HWDGE rings execute DMAs in
                FIFO order per ring, so sharing a ring with the y stores
                (sync) head-of-line-blocks them, and issuing on the scalar
                ring stalls ACT evictions. The Q7 ring is otherwise idle.
                """
                tiles = []
                for m in range(MSUB):
                    xm = xs.tile([128, K_SUB, 128], BF16, tag=f"x{m}",
                                 bufs=4, name=f"x{m}_{c}")
                    nc.gpsimd.dma_start(xm[:], x_d[c, :, m])
                    tiles.append(xm)
                return tiles

            def mm_group(c, m, n, xk):
                """One accumulation group + eviction + 256KB y store.

                Evictions split by n-slice across ACT and DVE so neither
                engine's W-phase work stalls PSUM bank recycling.
                """
                acc = ymm.tile([128, N_SLICE], F32, tag="y_ps",
                               name=f"acc_{c}_{m}_{n}", bufs=5)
                lhs = xk[m]
                for k in range(K_SUB):
                    nc.tensor.matmul(
                        acc[:],
                        lhs[:, k, :],
                        wts[n][:, k, :],
                        start=(k == 0),
                        stop=(k == K_SUB - 1),
                    )
                y_sb = ysp.tile([128, N_SLICE], F32, tag="y_sb",
                                name=f"y_sb_{c}_{m}_{n}", bufs=6)
                if n == 0:
                    nc.scalar.copy(y_sb[:], acc[:])
                else:
                    nc.vector.tensor_copy(y_sb[:], acc[:])
                nc.sync.dma_start(
                    y_d[(c * MSUB + m) * 128 : (c * MSUB + m + 1) * 128,
                        n * N_SLICE : (n + 1) * N_SLICE],
                    y_sb[:],
                )

            def ladder_og(acc, lhs, og):
                """One N=128 sub-group of the ladder: only needs W chunk
                og's wts[0] columns. Sub-groups share one PSUM bank on
                disjoint column ranges (start=True only clears
                has_written bits, not data, so earlier sub-groups' values
                survive)."""
                sl = slice(og * 128, (og + 1) * 128)
                for k in range(K_SUB):
                    nc.tensor.matmul(
                        acc[:, sl],
                        lhs[:, k, :],
                        wts[0][:, k, sl],
                        start=(k == 0),
                        stop=(k == K_SUB - 1),
                        skip_group_check=True,
                    )

            def ladder_end(c, m, acc):
                y_sb = ysp.tile([128, N_SLICE], F32, tag="y_sb",
                                name=f"ly_sb_{c}_{m}", bufs=6)
                nc.scalar.copy(y_sb[:], acc[:])
                nc.sync.dma_start(
                    y_d[(c * MSUB + m) * 128 : (c * MSUB + m + 1) * 128,
                        0:N_SLICE],
                    y_sb[:],
                )

            # Emission schedule. Lead-in critical path: w DMAs (two HWDGE
            # rings) -> ACT abs -> DVE/GPSIMD quant -> PE transpose -> ACT
            # evict, pipelined per W chunk; warm-up matmuls and the ladder
            # group keep the PE from idling (and the HAM clock gate from
            # re-throttling) while wts[0] completes. W chunks 4-7 are
            # quantized during chunk 0/1's n=0 groups, with their PE
            # transposes placed *after* chunk 1's groups so they never
            # head-of-line-block the matmul stream waiting on quant data.
            for a in range(4):
                w_dma(a)
            xk0 = x_load(0)
            warmup(10)
            w_quant(0)
            xk1 = x_load(1)
            lacc = ymm.tile([128, N_SLICE], F32, tag="y_ps",
                            name="lacc_0_0", bufs=5)
            ladder_og(lacc, xk0[0], 0)
            w_quant(1)
            ladder_og(lacc, xk0[0], 1)
            w_quant(2)
            ladder_og(lacc, xk0[0], 2)
            w_quant(3)
            ladder_og(lacc, xk0[0], 3)
            ladder_end(0, 0, lacc)
            for a in range(4, 8):
                w_dma(a)
            for m in range(1, MSUB):
                mm_group(0, m, 0, xk0)
            xk2 = x_load(2)
            for m in range(MSUB):
                mm_group(1, m, 0, xk1)
                w_quant(4 + m)
            for m in range(MSUB):
                mm_group(0, m, 1, xk0)
            xk3 = x_load(3)
            for m in range(MSUB):
                mm_group(1, m, 1, xk1)
            xk_tiles = {2: xk2, 3: xk3}
            for c in range(2, NCHUNK):
                if c + 2 < NCHUNK:
                    xk_tiles[c + 2] = x_load(c + 2)
                for m in range(MSUB):
                    mm_group(c, m, 0, xk_tiles[c])
                    mm_group(c, m, 1, xk_tiles[c])

    nc.compile()
    return nc


_NC_CACHE = None


def _get_nc():
    global _NC_CACHE
    if _NC_CACHE is None:
        _NC_CACHE = _build()
    return _NC_CACHE


def kernel(x: np.ndarray, weight: np.ndarray, _trace: bool = False):
    assert x.shape == (B, S, D_IN) and weight.shape == (D_OUT, D_IN)
    # Host layout prep: bf16 [chunk, k_in, msub, k_sub, row] so each
    # (chunk, msub) DMAs in as ready-to-use stationary tiles (k on
    # partitions), contiguous per partition.
    x_flat = np.asarray(x, dtype=np.float32).reshape(R, D_IN)
    xr = np.ascontiguousarray(
        x_flat.reshape(NCHUNK, MSUB, 128, K_SUB, 128)
        .transpose(0, 4, 1, 3, 2)
        .astype(ml_dtypes.bfloat16)
    )
    in_maps = [
        {
            "x": xr,
            "w": np.ascontiguousarray(
                weight[c * O : (c + 1) * O], dtype=np.float32
            ),
        }
        for c in range(NCORES)
    ]
    nc = _get_nc()
    res = run_bass_kernel_spmd(
        nc, in_maps, core_ids=list(range(NCORES)), trace=_trace
    )
    y = np.concatenate([res.results[c]["y"] for c in range(NCORES)], axis=1)
    out = y.reshape(B, S, D_OUT)
    if _trace:
        return out, res
    return out


# revision 26
# speedup vs baseline: 1.7324x; 1.7324x over previous
"""BitLinear (ternary weight quantization + linear) on 8 TRN2 NeuronCores.

y = x @ w_eff.T with w_eff = clip(round(w/scale), -1, 1) * scale,
scale = clamp(mean |w| per row, 1e-5).

Sharding: column-parallel — weight rows (out_features) split 8 ways; each
core computes y[:, shard] for the full x; host concatenates. Quantization
is per-output-row, so it is fully local to a shard.

v2 dataflow (vs the transpose-on-device baseline):
  * The host pre-permutes x into [16 chunks][128 k_in][16 k_sub][512 rows]
    so every x tile lands in SBUF already in stationary (k-major) layout —
    the device does ZERO x preprocessing (the baseline spent ~86us of PE
    time on 1024 PE transposes plus DVE/ACT copies for this).
  * The matmul runs in bf16: w_eff is ternary*scale (exact in bf16 up to a
    coherent 0.4% per-row scale rounding), x is cast fp32->bf16 in-flight
    by the SWDGE DMA. bf16 enables fast weight load so the per-matmul
    LDWEIGHTS (~187ns in fp32r) hides completely under the 213ns matmul.
  * W phase keeps the baseline's quantization math bit-identical (Abs
    row-sum on ACT, scale clamp, is_gt/is_lt ternary build on DVE): the
    jax reference's round() at the 0.5 boundary is reproduced exactly —
    a single flipped ternary weight costs 1.35e-2 absmax error, 2/3 of
    the 2e-2 budget.
  * Warm-up matmuls at kernel start bring the PE HAM clock gate to 8/8
    before the real matmul stream begins.

Per-core steady state: 16 row-chunks of 512; per chunk one 4MiB cast-DMA
in, then 8 accumulation groups (4 m-subtiles x 2 n-slices) of 16 matmuls
[128x128]@[128x512], ACT eviction, 256KB y DMA out per group.
"""

import ml_dtypes
import numpy as np

import concourse.bass as bass
import concourse.mybir as mybir
import concourse.tile as tile
from concourse import bacc
from concourse.bass_utils import run_bass_kernel_spmd
from concourse.masks import make_identity

F32 = mybir.dt.float32
F32R = mybir.dt.float32r
BF16 = mybir.dt.bfloat16

# Problem shape (hardcoded per contract)
B, S, D_IN, D_OUT = 4, 2048, 2048, 8192
NCORES = 8
R = B * S                 # 8192 rows of x
O = D_OUT // NCORES       # 1024 out features per core
K_SUB = D_IN // 128       # 16 contraction sub-tiles
O_TILES = O // 128        # 8 weight row-tiles per core
N_SLICE = 512             # psum bank width (fp32)
N_SLICES = O // N_SLICE   # 2
TGRP = 4                  # transposes batched per psum bank
RCHUNK = 512              # x rows per streamed chunk
NCHUNK = R // RCHUNK      # 16
MSUB = RCHUNK // 128      # 4
N_WARM = 20               # HAM warm-up matmuls


def _build():
    nc = bacc.Bacc(None, target_bir_lowering=False)

    # x: host-permuted bf16 [chunk, k_in, msub, k_sub, row] (full x, same
    # on all cores); one 512KiB sub-DMA per (chunk, msub) keeps
    # dependencies fine-grained so the first matmul group only waits for
    # its own slice. Shipping bf16 halves x HBM traffic (the early phase
    # is HBM-bound: w + x + y competing) and avoids the SWDGE cast path.
    x_d = nc.dram_tensor("x", [NCHUNK, 128, MSUB, K_SUB, 128], BF16,
                         kind="ExternalInput")
    w_d = nc.dram_tensor("w", [O, D_IN], F32, kind="ExternalInput")
    y_d = nc.dram_tensor("y", [R, O], F32, kind="ExternalOutput")

    with tile.TileContext(nc) as tc:
        with (
            tc.tile_pool(name="const", bufs=1) as const,
            tc.tile_pool(name="wt", bufs=1) as wtp,
            tc.tile_pool(name="ws", bufs=1) as ws,
            tc.tile_pool(name="xs", bufs=1) as xs,
            tc.tile_pool(name="ys", bufs=1) as ysp,
            tc.tile_pool(name="ps", bufs=2, space="PSUM") as ps,
            tc.tile_pool(name="ymm", bufs=1, space="PSUM") as ymm,
        ):
            # HAM warm-up: keep the PE busy with throwaway matmuls during
            # the W-phase lead-in so the clock gate is at 8/8 when the
            # real stream starts. (PE transposes don't count as HAM-busy.)
            dummy = const.tile([128, N_SLICE], BF16)
            nc.vector.memset(dummy[:], 0.0)
            wacc = ymm.tile([128, N_SLICE], F32, tag="warm", bufs=1)

            def warmup(n):
                for _ in range(n):
                    nc.tensor.matmul(wacc[:], dummy[:, :128], dummy[:],
                                     start=True, stop=True)

            ident_f = const.tile([128, 128], F32)
            make_identity(nc, ident_f[:])
            ident = const.tile([128, 128], F32R)
            nc.vector.tensor_copy(ident[:], ident_f[:])

            # W^T resident in SBUF (bf16), one tile per n-slice:
            # wts[n][:, k, o'] = w_eff^T[k_in, k_sub, n*512 + o']
            wts = [
                wtp.tile([128, K_SUB, N_SLICE], BF16, name=f"wt{n}")
                for n in range(N_SLICES)
            ]

            w_tiles = {}

            def w_dma(a):
                """Start the DMA for weight rows a*128..(a+1)*128.

                Alternates between the two HWDGE rings so the 8 transfers
                overlap pairwise (each ring executes its DMAs in FIFO).
                """
                w_in = ws.tile([128, D_IN], F32, tag="w_in", bufs=4,
                               name=f"w_in_{a}")
                eng = nc.sync if a % 2 == 0 else nc.scalar
                eng.dma_start(w_in[:], w_d[a * 128 : (a + 1) * 128, :])
                w_tiles[a] = w_in

            def w_quant(a):
                """Quantize + transpose weight rows a*128..(a+1)*128.

                Math is bit-identical to the baseline (matches the jax
                reference's round-half behavior at the 0.5 boundary); only
                the final PSUM->SBUF eviction casts to bf16.
                """
                w_in = w_tiles.pop(a)

                absdump = ws.tile([128, D_IN], F32, tag="w_neg",
                                  name=f"absdump_{a}")
                ssum = ws.tile([128, 1], F32, tag="w_sum", name=f"ssum_{a}")
                nc.scalar.activation(
                    absdump[:], w_in[:],
                    mybir.ActivationFunctionType.Abs,
                    accum_out=ssum[:],
                )
                scale = ws.tile([128, 1], F32, tag="w_scale",
                                name=f"scale_{a}")
                nc.vector.tensor_scalar(
                    out=scale[:], in0=ssum[:], scalar1=1.0 / D_IN,
                    scalar2=1e-5, op0=mybir.AluOpType.mult,
                    op1=mybir.AluOpType.max,
                )
                hpos = ws.tile([128, 1], F32, tag="w_hpos", name=f"hp_{a}")
                hneg = ws.tile([128, 1], F32, tag="w_hneg", name=f"hn_{a}")
                nc.vector.tensor_scalar_mul(hpos[:], scale[:], 0.5)
                nc.vector.tensor_scalar_mul(hneg[:], scale[:], -0.5)

                # (w > 0.5*scale)*scale - (w < -0.5*scale)*scale
                pos = ws.tile([128, D_IN], F32, tag="w_pos", name=f"pos_{a}")
                nc.vector.tensor_scalar(
                    out=pos[:], in0=w_in[:], scalar1=hpos[:], scalar2=scale[:],
                    op0=mybir.AluOpType.is_gt, op1=mybir.AluOpType.mult,
                )
                neg = ws.tile([128, D_IN], F32, tag="w_neg", name=f"neg_{a}")
                nc.vector.tensor_scalar(
                    out=neg[:], in0=w_in[:], scalar1=hneg[:], scalar2=scale[:],
                    op0=mybir.AluOpType.is_lt, op1=mybir.AluOpType.mult,
                )
                weff = ws.tile([128, D_IN], F32R, tag="w_eff",
                               name=f"weff_{a}")
                nc.vector.tensor_sub(weff[:], pos[:], neg[:])

                n_idx, o_off = divmod(a * 128, N_SLICE)
                for kg in range(K_SUB // TGRP):
                    pt = ps.tile([128, TGRP * 128], F32, tag="wtps", bufs=2,
                                 name=f"wpt_{a}_{kg}")
                    for j in range(TGRP):
                        k = kg * TGRP + j
                        nc.tensor.transpose(
                            pt[:, j * 128 : (j + 1) * 128].bitcast(F32R),
                            weff[:, k * 128 : (k + 1) * 128],
                            ident[:],
                        )
                    half = TGRP // 2
                    dst = wts[n_idx][:, kg * TGRP : (kg + 1) * TGRP,
                                     o_off : o_off + 128]
                    src = pt[:].rearrange("p (g c) -> p g c", g=TGRP)
                    nc.scalar.copy(dst[:, :half], src[:, :half])
                    nc.scalar.copy(dst[:, half:], src[:, half:])

            def x_load(c):
                """Start 4 per-msub SWDGE DMAs for x chunk c (512KiB each).

                On the gpsimd (SWDGE) path: HWDGE rings execute DMAs in
                FIFO order per ring, so sharing a ring with the y stores
                (sync) head-of-line-blocks them, and issuing on the scalar
                ring stalls ACT evictions. The Q7 ring is otherwise idle.
                """
                tiles = []
                for m in range(MSUB):
                    xm = xs.tile([128, K_SUB, 128], BF16, tag=f"x{m}",
                                 bufs=4, name=f"x{m}_{c}")
                    nc.gpsimd.dma_start(xm[:], x_d[c, :, m])
                    tiles.append(xm)
                return tiles

            def mm_group(c, m, n, xk):
                """One accumulation group + eviction + 256KB y store.

                Evictions split by n-slice across ACT and DVE so neither
                engine's W-phase work stalls PSUM bank recycling.
                """
                acc = ymm.tile([128, N_SLICE], F32, tag="y_ps",
                               name=f"acc_{c}_{m}_{n}", bufs=5)
                lhs = xk[m]
                for k in range(K_SUB):
                    nc.tensor.matmul(
                        acc[:],
                        lhs[:, k, :],
                        wts[n][:, k, :],
                        start=(k == 0),
                        stop=(k == K_SUB - 1),
                    )
                y_sb = ysp.tile([128, N_SLICE], F32, tag="y_sb",
                                name=f"y_sb_{c}_{m}_{n}", bufs=6)
                if n == 0:
                    nc.scalar.copy(y_sb[:], acc[:])
                else:
                    nc.vector.tensor_copy(y_sb[:], acc[:])
                nc.sync.dma_start(
                    y_d[(c * MSUB + m) * 128 : (c * MSUB + m + 1) * 128,
                        n * N_SLICE : (n + 1) * N_SLICE],
                    y_sb[:],
                )

            def ladder_og(acc, lhs, og):
                """One N=128 sub-group of the ladder: only needs W chunk
                og's wts[0] columns. Sub-groups share one PSUM bank on
                disjoint column ranges (start=True only clears
                has_written bits, not data, so earlier sub-groups' values
                survive)."""
                sl = slice(og * 128, (og + 1) * 128)
                for k in range(K_SUB):
                    nc.tensor.matmul(
                        acc[:, sl],
                        lhs[:, k, :],
                        wts[0][:, k, sl],
                        start=(k == 0),
                        stop=(k == K_SUB - 1),
                        skip_group_check=True,
                    )

            def ladder_end(c, m, acc):
                y_sb = ysp.tile([128, N_SLICE], F32, tag="y_sb",
                                name=f"ly_sb_{c}_{m}", bufs=6)
                nc.scalar.copy(y_sb[:], acc[:])
                nc.sync.dma_start(
                    y_d[(c * MSUB + m) * 128 : (c * MSUB + m + 1) * 128,
                        0:N_SLICE],
                    y_sb[:],
                )

            # Emission schedule. Lead-in critical path: w DMAs (two HWDGE
            # rings) -> ACT abs -> DVE/GPSIMD quant -> PE transpose -> ACT
            # evict, pipelined per W chunk; warm-up matmuls and the ladder
            # group keep the PE from idling (and the HAM clock gate from
            # re-throttling) while wts[0] completes. W chunks 4-7 are
            # quantized during chunk 0/1's n=0 groups, with their PE
            # transposes placed *after* chunk 1's groups so they never
            # head-of-line-block the matmul stream waiting on quant data.
            for a in range(4):
                w_dma(a)
            xk0 = x_load(0)
            warmup(10)
            w_quant(0)
            xk1 = x_load(1)
            lacc = ymm.tile([128, N_SLICE], F32, tag="y_ps",
                            name="lacc_0_0", bufs=5)
            ladder_og(lacc, xk0[0], 0)
            w_quant(1)
            ladder_og(lacc, xk0[0], 1)
            w_quant(2)
            ladder_og(lacc, xk0[0], 2)
            w_quant(3)
            ladder_og(lacc, xk0[0], 3)
            ladder_end(0, 0, lacc)
            for a in range(4, 8):
                w_dma(a)
            for m in range(1, MSUB):
                mm_group(0, m, 0, xk0)
            xk2 = x_load(2)
            for m in range(MSUB):
                mm_group(1, m, 0, xk1)
                w_quant(4 + m)
            for m in range(MSUB):
                mm_group(0, m, 1, xk0)
            xk3 = x_load(3)
            for m in range(MSUB):
                mm_group(1, m, 1, xk1)
            xk_tiles = {2: xk2, 3: xk3}
            for c in range(2, NCHUNK):
                if c + 2 < NCHUNK:
                    xk_tiles[c + 2] = x_load(c + 2)
                for m in range(MSUB):
                    mm_group(c, m, 0, xk_tiles[c])
                    mm_group(c, m, 1, xk_tiles[c])

    nc.compile()
    return nc


_NC_CACHE = None


def _get_nc():
    global _NC_CACHE
    if _NC_CACHE is None:
        _NC_CACHE = _build()
    return _NC_CACHE


def kernel(x: np.ndarray, weight: np.ndarray, _trace: bool = False):
    assert x.shape == (B, S, D_IN) and weight.shape == (D_OUT, D_IN)
    # Host layout prep: bf16 [chunk, k_in, msub, k_sub, row] so each
    # (chunk, msub) DMAs in as ready-to-use stationary tiles (k on
    # partitions), contiguous per partition.
    x_flat = np.asarray(x, dtype=np.float32).reshape(R, D_IN)
    xr = np.ascontiguousarray(
        x_flat.reshape(NCHUNK, MSUB, 128, K_SUB, 128)
        .transpose(0, 4, 1, 3, 2)
        .astype(ml_dtypes.bfloat16)
    )
    in_maps = [
        {
            "x": xr,
            "w": np.ascontiguousarray(
                weight[c * O : (c + 1) * O], dtype=np.float32
            ),
        }
        for c in range(NCORES)
    ]
    nc = _get_nc()
    res = run_bass_kernel_spmd(
        nc, in_maps, core_ids=list(range(NCORES)), trace=_trace
    )
    y = np.concatenate([res.results[c]["y"] for c in range(NCORES)], axis=1)
    out = y.reshape(B, S, D_OUT)
    if _trace:
        return out, res
    return out
